# revision 29
# baseline (speedup 1.0000x reference)
"""Multi-head attention (B=2, L=S=2048, D=1024, H=16) on 8 Trainium2 cores.

Sharding: core c -> batch b = c // 4, head group g = c % 4 (4 heads per core).
W_Q/K/V column-sharded (256 cols per core), W_O row-sharded (256 rows per core);
the 4 partial outputs per batch are summed on the host (plus bias terms).

Per-core pipeline (all big tensors kept transposed so no on-device transposes):
  projections: QT = 0.125*(x Wq + bq)^T, KT = (x Wk + bk)^T (feature-major
    [256, L]); Vaug = [V_h | ones] per head (seq-major, fp16), V bias folded
    out on the host (softmax rows sum to 1 => + bv @ Wo + bo once).
  attention, per (l-tile 512, s-tile 128): S^T = KT^T QT (row-packed pairs of
    heads, K=64); E = exp(S^T) * maskT (ACT exp from PSUM, per-pair 0/1 fp16
    mask multiply on DVE at 2x); T_h += Vaug_h^T E accumulates BOTH the head
    output AND its softmax row-sums in one full-array matmul (ones columns
    act as the reducer; even heads get [V|1] -> av in rows 0:64, odd heads
    [1|V] -> av in rows 64:128 so every result lands on the lanes the
    output-projection layout needs).
  out-projection: out_partial = outT^T Wo_rows (K=128, accumulate over the
    two 128-row groups).

Scheduling (the kernel is EXP-gated in steady state at ~1us/exp, with the
PE ~90% subscribed inside each period, so projections must NOT be pushed
into the attention stream):
  - serial prefix: all of KT/V/QT(l-tile 0), with x DMA'd in l-quarters so
    the PE starts ~4us in, plus ~40 tiny warm-up matmuls so the HAM clock
    gate reaches 8/8 before the real projections;
  - QT for l-tile lt+1 is the only producer injected mid-stream (2x 8
    matmuls borrowing the score PSUM rotation);
  - the per-l-tile softmax normalization is split into 3 phases injected
    into the NEXT l-tile's first s-iterations: phase 1 reads both halves of
    each T bank out of PSUM early (av halves on ACT, which idles at the
    boundary; row sums via DVE recip/copy) so the banks release ~1.5us
    after the last aug matmul; lane swaps (reciprocal_approx_fast only
    works at partition base 0) and the final multiplies run off the
    critical path on SBUF staging;
  - tail: the first 4 output-projection groups go through the score PSUM
    slots, which the final exps release BEFORE the last aug matmuls finish
    (they overlap the end of the stream and the normalization chain); then
    phase 1 of the last l-tile releases the T banks and the remaining 12
    groups stream through them at single-bank granularity (4-deep rotation;
    ACT/DVE alternate the PSUM evacuations, both idle after the last exp).

All matmul operands fp16 (1 cyc/row, no packing restrictions); PSUM fp32.
PSUM budget 8 banks = scores 2x2 + T_h 4x1; projections borrow the score
slots, the tail out-projection borrows the T banks.
"""
from contextlib import ExitStack

import numpy as np

import concourse.bass as bass
import concourse.mybir as mybir
import concourse.tile as tile
from concourse import bacc
from concourse.bass_utils import run_bass_kernel_spmd

F16 = mybir.dt.float16
F32 = mybir.dt.float32

D = 1024          # d_model
H = 16            # heads
DK = 64           # head dim
B, L = 2, 2048
NCORES = 8
HPC = 4           # heads per core
FPC = HPC * DK    # features per core = 256
KD = D // 128     # 8 contraction subtiles for projections
LT, LTW = 4, 512  # l tiles
ST, STW = 16, 128  # s tiles
MPF = 3           # mask DMA prefetch depth
Ident = mybir.ActivationFunctionType.Identity
Exp = mybir.ActivationFunctionType.Exp

_CACHED_NC = None


def _build():
    nc = bacc.Bacc("TRN2", target_bir_lowering=False, debug=False,
                   num_devices=NCORES)
    xT = nc.declare_dram_parameter("xT", [128, KD, L], F16, isOutput=False)
    wq = nc.declare_dram_parameter("wq", [128, KD, FPC], F16, isOutput=False)
    wk = nc.declare_dram_parameter("wk", [128, KD, FPC], F16, isOutput=False)
    wv = nc.declare_dram_parameter("wv", [128, KD, FPC], F16, isOutput=False)
    wo = nc.declare_dram_parameter("wo", [128, 2, D], F16, isOutput=False)
    bq = nc.declare_dram_parameter("bq", [128, 2], F32, isOutput=False)
    bk = nc.declare_dram_parameter("bk", [128, 2], F32, isOutput=False)
    maskT = nc.declare_dram_parameter("maskT", [ST, LT, 128, LTW], F16,
                                      isOutput=False)
    out = nc.declare_dram_parameter("out", [128, ST, D], F16, isOutput=True)

    with tile.TileContext(nc) as tc, ExitStack() as ctx:
        pool = ctx.enter_context(tc.tile_pool(name="pers", bufs=1))
        mpool = ctx.enter_context(tc.tile_pool(name="mpool", bufs=2 * MPF))
        epool = ctx.enter_context(tc.tile_pool(name="epool", bufs=4))
        rbpool = ctx.enter_context(tc.tile_pool(name="rbpool", bufs=1))
        avpool = ctx.enter_context(tc.tile_pool(name="avpool", bufs=1))
        opool = ctx.enter_context(tc.tile_pool(name="opool", bufs=3))
        scp = ctx.enter_context(tc.tile_pool(name="scp", bufs=2, space="PSUM"))
        tp = ctx.enter_context(tc.tile_pool(name="tp", bufs=1, space="PSUM"))

        xt = pool.tile([128, KD, L], F16)
        wq_sb = pool.tile([128, KD, FPC], F16)
        wk_sb = pool.tile([128, KD, FPC], F16)
        wv_sb = pool.tile([128, KD, FPC], F16)
        wo_sb = pool.tile([128, 2, D], F16)
        bq_sb = pool.tile([128, 2], F32)
        bk_sb = pool.tile([128, 2], F32)
        # DMA issue order follows the dependency order of the first
        # matmuls.  x is transferred in l-quarters: the whole prefix (KT
        # chunk 0, QT l-chunk 0, V s-tiles 0-3) only touches sequence
        # positions 0:512, so the PE can start ~3.5us in instead of
        # waiting ~12us for all of x.
        nc.sync.dma_start(out=wk_sb[:], in_=wk[:])
        for kd in range(KD):
            nc.sync.dma_start(out=xt[:, kd, 0:LTW], in_=xT[:, kd, 0:LTW])
        nc.sync.dma_start(out=wv_sb[:], in_=wv[:])
        nc.sync.dma_start(out=wq_sb[:], in_=wq[:])
        nc.sync.dma_start(out=bk_sb[:], in_=bk[:])
        nc.sync.dma_start(out=bq_sb[:], in_=bq[:])
        for q in range(1, LT):
            qsl = slice(q * LTW, (q + 1) * LTW)
            for kd in range(KD):
                nc.sync.dma_start(out=xt[:, kd, qsl], in_=xT[:, kd, qsl])
        nc.sync.dma_start(out=wo_sb[:], in_=wo[:])

        # PE warmup: ~64 tiny matmuls on a zeroed tile keep the PE busy
        # during the initial DMA wait so the HAM clock-gate reaches 8/8
        # before the real projection matmuls start (cold MMs run at half
        # clock for the first ~3.4us of activity otherwise).
        wu_sb = pool.tile([128, 64], F16)
        nc.vector.memset(wu_sb[:], 0.0)
        wu_ps = scp.tile([128, 2, LTW], F32, tag="sc", name="wu")
        for i in range(40):
            nc.tensor.matmul(wu_ps[0:64, 0, 0:64], wu_sb[:], wu_sb[:],
                             start=True, stop=True)

        QT = pool.tile([128, 2, L], F16)   # [feat(2x128), l]: Q^T * 0.125
        KT = pool.tile([128, 2, L], F16)
        # Vaug[:, st, h]: even h -> [V_h | 1], odd h -> [1 | V_h]
        Vaug = pool.tile([128, ST, HPC, 128], F16)
        nc.gpsimd.memset(Vaug[:], 1.0)
        outTs = [pool.tile([128, 2, LTW], F16, name=f"outT{i}")
                 for i in range(LT)]

        # ---- producers, emitted in small quanta between attention s-tile
        # ---- iterations so the PE absorbs them while ACT (exp) streams.
        def emit_kt_half(c, ft):
            lsl = slice(c * LTW, (c + 1) * LTW)
            fsl = slice(ft * 128, (ft + 1) * 128)
            ps = scp.tile([128, 2, LTW], F32, tag="sc", name=f"pk{c}_{ft}")
            for kd in range(KD):
                nc.tensor.matmul(ps[:, 0, :], wk_sb[:, kd, fsl],
                                 xt[:, kd, lsl],
                                 start=(kd == 0), stop=(kd == KD - 1))
            nc.vector.scalar_tensor_tensor(
                KT[:, ft, lsl], ps[:, 0, :], 1.0,
                bk_sb[:, ft:ft + 1].to_broadcast((128, LTW)),
                mybir.AluOpType.mult, mybir.AluOpType.add)

        def emit_qt_half(lt, ft):
            lsl = slice(lt * LTW, (lt + 1) * LTW)
            fsl = slice(ft * 128, (ft + 1) * 128)
            ps = scp.tile([128, 2, LTW], F32, tag="sc", name=f"pq{lt}_{ft}")
            for kd in range(KD):
                nc.tensor.matmul(ps[:, 0, :], wq_sb[:, kd, fsl],
                                 xt[:, kd, lsl],
                                 start=(kd == 0), stop=(kd == KD - 1))
            nc.vector.scalar_tensor_tensor(
                QT[:, ft, lsl], ps[:, 0, :], 0.125,
                bq_sb[:, ft:ft + 1].to_broadcast((128, LTW)),
                mybir.AluOpType.mult, mybir.AluOpType.add)

        def emit_v_chunk(c):
            for st in range(4 * c, 4 * c + 4):
                ssl = slice(st * STW, (st + 1) * STW)
                psv = tp.tile([128, LTW], F32, tag=f"T{st % 4}",
                              name=f"psv{st}")
                for kd in range(KD):
                    nc.tensor.matmul(psv[:, :FPC], xt[:, kd, ssl],
                                     wv_sb[:, kd, :],
                                     start=(kd == 0), stop=(kd == KD - 1))
                for h in range(HPC):
                    off = 0 if h % 2 == 0 else 64
                    nc.vector.tensor_copy(Vaug[:, st, h, off:off + 64],
                                          psv[:, DK * h:DK * (h + 1)])

        def emit_outproj_group(lt8):
            ps3 = scp.tile([128, 2, LTW], F32, tag="sc", name=f"ps3_{lt8}")
            for nf in range(2):
                nsl = slice(nf * 512, (nf + 1) * 512)
                for pair in range(2):
                    nc.tensor.matmul(
                        ps3[:, nf, :],
                        outTs[lt8 // 4][:, pair,
                                        (lt8 % 4) * 128:(lt8 % 4 + 1) * 128],
                        wo_sb[:, pair, nsl],
                        start=(pair == 0), stop=(pair == 1))
            ob = opool.tile([128, D], F16)
            # evacuate the two PSUM banks on ACT and DVE in parallel so the
            # score-slot rotation frees ~2x sooner (tail pace is gated on it)
            nc.scalar.copy(ob[:, 0:512], ps3[:, 0, :])
            nc.vector.tensor_copy(ob[:, 512:1024], ps3[:, 1, :])
            nc.gpsimd.dma_start(out=out[:, lt8, :], in_=ob[:])

        # Injected work: QT for l-tile lt+1 is produced mid-lt (so no PE
        # burst at the boundary).  Everything else (KT, V) is produced in
        # the serial prefix below -- the attention phase has no PE slack to
        # absorb it (EXP-gated periods are ~95% PE-occupied already).
        inject = {}
        for lt in range(LT - 1):
            inject[(lt, 7)] = [lambda lt=lt: emit_qt_half(lt + 1, 0)]
            inject[(lt, 9)] = [lambda lt=lt: emit_qt_half(lt + 1, 1)]

        # ---- serial prefix: all projections, ordered to match DMA arrival
        # ---- of the x l-quarters (chunk c needs quarter c only).
        for c in range(4):
            emit_kt_half(c, 0)
            emit_kt_half(c, 1)
            if c == 0:
                emit_qt_half(0, 0)
                emit_qt_half(0, 1)
            emit_v_chunk(c)

        mk_tiles = {}
        mk_order = [(lt, st) for lt in range(LT) for st in range(ST)]

        def prefetch_mask(pos):
            if pos < len(mk_order):
                plt, pst = mk_order[pos]
                mk = mpool.tile([128, LTW], F16)
                nc.sync.dma_start(out=mk[:], in_=maskT[pst, plt])
                mk_tiles[(plt, pst)] = mk

        for pos in range(MPF):
            prefetch_mask(pos)

        # Per-head softmax normalization, split into three phases that are
        # injected into the NEXT l-tile's first s-iterations so the Tile
        # scheduler orders them after that l-tile's score/exp stream.  The
        # T PSUM banks are released by phase 1 (both halves of each bank
        # read out: av halves via ACT, which idles at the boundary, row
        # sums via DVE); the lane swaps, reciprocals and final multiplies
        # run off the critical path on SBUF staging tiles.
        rbs = [rbpool.tile([128, LTW], F32, name=f"rb{h}")
               for h in range(HPC)]
        avs = [avpool.tile([128, LTW], F32, name=f"av{h}")
               for h in range(HPC)]

        def chain_phase1a(Ts, on_act=True):
            for h in (1, 3):   # odd: av at 64:128, sums at 0:64
                nc.vector.reciprocal_approx_fast(out=rbs[h][0:64, :],
                                                 in_=Ts[h][0:64, :])
                if on_act:
                    nc.scalar.copy(avs[h][64:128, :], Ts[h][64:128, :])
                else:
                    nc.vector.tensor_copy(avs[h][64:128, :],
                                          Ts[h][64:128, :])

        def chain_phase1b(Ts, on_act=True):
            for h in (0, 2):   # even: av at 0:64, sums at 64:128
                if on_act:
                    nc.scalar.copy(avs[h][0:64, :], Ts[h][0:64, :])
                else:
                    nc.vector.tensor_copy(avs[h][0:64, :], Ts[h][0:64, :])
                nc.vector.tensor_copy(rbs[h][64:128, :], Ts[h][64:128, :])

        def chain_phase2a():
            # gpsimd queue: idle mid-stream, while Sync is busy with the
            # mask prefetches
            for h in (1, 3):
                nc.gpsimd.dma_start(out=rbs[h][64:128, :],
                                    in_=rbs[h][0:64, :])

        def chain_phase2b():
            for h in (0, 2):
                nc.gpsimd.dma_start(out=rbs[h][0:64, :],
                                    in_=rbs[h][64:128, :])

        def chain_phase1(Ts, on_act=True):
            chain_phase1a(Ts, on_act)
            chain_phase1b(Ts, on_act)

        def chain_phase2():
            chain_phase2a()
            chain_phase2b()

        def chain_phase3(lt):
            for h in (1, 3):
                nc.vector.tensor_mul(outTs[lt][64:128, h // 2, :],
                                     avs[h][64:128, :], rbs[h][64:128, :])
            for h in (0, 2):
                nc.vector.reciprocal_approx_fast(out=rbs[h][0:64, :],
                                                 in_=rbs[h][0:64, :])
                nc.vector.tensor_mul(outTs[lt][0:64, h // 2, :],
                                     avs[h][0:64, :], rbs[h][0:64, :])

        prevTs = None
        for lt in range(LT):
            lsl = slice(lt * LTW, (lt + 1) * LTW)
            Ts = [tp.tile([128, LTW], F32, tag=f"T{h}", name=f"T{h}_{lt}")
                  for h in range(HPC)]
            for st in range(ST):
                prefetch_mask(lt * ST + st + MPF)
                ssl = slice(st * STW, (st + 1) * STW)
                mk = mk_tiles.pop((lt, st))
                Es = []
                for pair in range(2):
                    sc = scp.tile([128, 2, LTW], F32, tag="sc")
                    for i in range(2):
                        nc.tensor.matmul(
                            sc[:, i, :],
                            KT[64 * i:64 * (i + 1), pair, ssl],
                            QT[64 * i:64 * (i + 1), pair, lsl],
                            start=True, stop=True)
                    E = epool.tile([128, 2, LTW], F16, name=f"E{pair}")
                    nc.scalar.activation(E[:], sc[:], Exp)
                    nc.vector.tensor_mul(
                        E[:], E[:],
                        mk[:, None, :].to_broadcast((128, 2, LTW)))
                    Es.append(E)
                # injected work sits between the score and aug matmuls in
                # emission (priority) order, filling the PE wait for the
                # exp+mask chain of this s-tile.
                if prevTs is not None:
                    if st == 0:
                        chain_phase1(prevTs, on_act=False)
                    elif st == 1:
                        chain_phase2()
                    elif st == 2:
                        chain_phase3(lt - 1)
                for fn in inject.get((lt, st), ()):
                    fn()
                # all four aug matmuls back-to-back: one weight-swap drain
                # boundary per s-tile instead of one per pair.  At st==0 the
                # odd heads go first: their T banks are released first by the
                # previous l-tile's phase-1 extraction.
                h_order = (1, 3, 0, 2) if st == 0 else (0, 1, 2, 3)
                for h in h_order:
                    pair, i = divmod(h, 2)
                    nc.tensor.matmul(Ts[h][:], Vaug[:, st, h, :],
                                     Es[pair][:, i, :],
                                     start=(st == 0), stop=(st == ST - 1))
            prevTs = Ts

        # tail: release the T banks immediately (phase 1), then stream all
        # 16 output-projection groups through them at single-bank
        # granularity (4-deep rotation, ACT/DVE alternating evacuations);
        # the rest of the last normalization (lane swaps, reciprocals,
        # multiplies) runs underneath the first groups and only the last 4
        # groups depend on it.
        # The first four groups go through the score PSUM slots, which are
        # released by the final exps BEFORE the last aug matmuls finish --
        # they overlap the end of the stream and the normalization chain.
        for lt8 in range(4):
            emit_outproj_group(lt8)
        chain_phase1(prevTs)

        def tail_outproj(lt8):
            ob = opool.tile([128, D], F16)
            for nf in range(2):
                nsl = slice(nf * 512, (nf + 1) * 512)
                pso = tp.tile([128, LTW], F32, tag=f"T{(2 * lt8 + nf) % 4}",
                              name=f"pso{lt8}_{nf}")
                for pair in range(2):
                    nc.tensor.matmul(
                        pso[:, :],
                        outTs[lt8 // 4][:, pair,
                                        (lt8 % 4) * 128:(lt8 % 4 + 1) * 128],
                        wo_sb[:, pair, nsl],
                        start=(pair == 0), stop=(pair == 1))
                if nf == 0:
                    nc.scalar.copy(ob[:, nsl], pso[:, :])
                else:
                    nc.vector.tensor_copy(ob[:, nsl], pso[:, :])
            if lt8 % 2 == 0:
                nc.gpsimd.dma_start(out=out[:, lt8, :], in_=ob[:])
            else:
                nc.sync.dma_start(out=out[:, lt8, :], in_=ob[:])

        for lt8 in range(4, 8):
            tail_outproj(lt8)
        chain_phase2()
        for lt8 in range(8, 12):
            tail_outproj(lt8)
        chain_phase3(LT - 1)
        for lt8 in range(12, 4 * LT):
            tail_outproj(lt8)

    nc.compile()
    return nc


def _get_nc():
    global _CACHED_NC
    if _CACHED_NC is None:
        _CACHED_NC = _build()
    return _CACHED_NC


def _prep_core_inputs(c, x, mask, Wq, bq, Wk, bk, Wv, Wo):
    b, g = divmod(c, 4)
    cs = slice(g * FPC, (g + 1) * FPC)

    xT = np.ascontiguousarray(
        x[b].T.reshape(KD, 128, L).transpose(1, 0, 2)).astype(np.float16)
    wq_c = np.ascontiguousarray(
        Wq[:, cs].reshape(KD, 128, FPC).transpose(1, 0, 2)).astype(np.float16)
    wk_c = np.ascontiguousarray(
        Wk[:, cs].reshape(KD, 128, FPC).transpose(1, 0, 2)).astype(np.float16)
    wv_c = np.ascontiguousarray(
        Wv[:, cs].reshape(KD, 128, FPC).transpose(1, 0, 2)).astype(np.float16)
    wo_c = np.ascontiguousarray(
        Wo[cs, :].reshape(2, 128, D).transpose(1, 0, 2)).astype(np.float16)
    bq_c = np.ascontiguousarray(
        (bq[cs] * 0.125).reshape(2, 128).T).astype(np.float32)
    bk_c = np.ascontiguousarray(bk[cs].reshape(2, 128).T).astype(np.float32)
    mT = mask[b].astype(np.float16).T  # [S, L]
    maskT = np.ascontiguousarray(
        mT.reshape(ST, 128, LT, LTW).transpose(0, 2, 1, 3))
    return {"xT": xT, "wq": wq_c, "wk": wk_c, "wv": wv_c, "wo": wo_c,
            "bq": bq_c, "bk": bk_c, "maskT": maskT}


def kernel(x, mask, Wq, bq, Wk, bk, Wv, bv, Wo, bo):
    x = np.asarray(x, np.float32)
    mask = np.asarray(mask)
    Wq, bq = np.asarray(Wq, np.float32), np.asarray(bq, np.float32)
    Wk, bk = np.asarray(Wk, np.float32), np.asarray(bk, np.float32)
    Wv, bv = np.asarray(Wv, np.float32), np.asarray(bv, np.float32)
    Wo, bo = np.asarray(Wo, np.float32), np.asarray(bo, np.float32)

    nc = _get_nc()
    in_maps = [_prep_core_inputs(c, x, mask, Wq, bq, Wk, bk, Wv, Wo)
               for c in range(NCORES)]
    res = run_bass_kernel_spmd(nc, in_maps, list(range(NCORES)))

    const_vec = (bv @ Wo + bo).astype(np.float32)  # A rows sum to 1
    outs = []
    for b in range(B):
        acc = np.zeros((L, D), np.float32)
        for g in range(4):
            part = res.results[4 * b + g]["out"]  # [128, 16, 1024] fp16
            acc += part.transpose(1, 0, 2).reshape(L, D).astype(np.float32)
        acc += const_vec
        outs.append(acc)
    return np.stack(outs)


# revision 30
# speedup vs baseline: 1.0443x; 1.0443x over previous
"""Multi-head attention (B=2, L=S=2048, D=1024, H=16) on 8 Trainium2 cores.

Sharding: core c -> batch b = c // 4, head group g = c % 4 (4 heads per core).
W_Q/K/V column-sharded (256 cols per core), W_O row-sharded (256 rows per core);
the 4 partial outputs per batch are summed on the host (plus bias terms).

Per-core pipeline (all big tensors kept transposed so no on-device transposes):
  projections: QT = 0.125*(x Wq + bq)^T, KT = (x Wk + bk)^T (feature-major
    [256, L]); Vaug = [V_h | ones] per head (seq-major, fp16), V bias folded
    out on the host (softmax rows sum to 1 => + bv @ Wo + bo once).
  attention, per (l-tile 512, s-tile 128): S^T = KT^T QT (row-packed pairs of
    heads, K=64); E = exp(S^T) * maskT (ACT exp from PSUM, per-pair 0/1 fp16
    mask multiply on DVE at 2x); T_h += Vaug_h^T E accumulates BOTH the head
    output AND its softmax row-sums in one full-array matmul (ones columns
    act as the reducer; even heads get [V|1] -> av in rows 0:64, odd heads
    [1|V] -> av in rows 64:128 so every result lands on the lanes the
    output-projection layout needs).
  out-projection: out_partial = outT^T Wo_rows (K=128, accumulate over the
    two 128-row groups).

Scheduling (the kernel is EXP-gated in steady state at ~1us/exp, with the
PE ~90% subscribed inside each period, so projections must NOT be pushed
into the attention stream):
  - serial prefix: all of KT/V/QT(l-tile 0), with x DMA'd in l-quarters so
    the PE starts ~4us in, plus ~40 tiny warm-up matmuls so the HAM clock
    gate reaches 8/8 before the real projections;
  - QT for l-tile lt+1 is the only producer injected mid-stream (2x 8
    matmuls borrowing the score PSUM rotation);
  - the per-l-tile softmax normalization is split into 3 phases injected
    into the NEXT l-tile's first s-iterations: phase 1 reads both halves of
    each T bank out of PSUM early (av halves on ACT, which idles at the
    boundary; row sums via DVE recip/copy) so the banks release ~1.5us
    after the last aug matmul; lane swaps (reciprocal_approx_fast only
    works at partition base 0) and the final multiplies run off the
    critical path on SBUF staging;
  - tail: phase 1 of the last l-tile releases the T banks, then all 16
    output-projection groups stream through them at single-bank granularity
    (4-deep rotation; ACT/DVE alternate the PSUM evacuations, both idle
    after the last exp).

All matmul operands fp16 (1 cyc/row, no packing restrictions); PSUM fp32.
PSUM budget 8 banks = scores 2x2 + T_h 4x1; projections borrow the score
slots, the tail out-projection borrows the T banks.
"""
from contextlib import ExitStack

import numpy as np

import concourse.bass as bass
import concourse.mybir as mybir
import concourse.tile as tile
from concourse import bacc
from concourse.bass_utils import run_bass_kernel_spmd

F16 = mybir.dt.float16
F32 = mybir.dt.float32

D = 1024          # d_model
H = 16            # heads
DK = 64           # head dim
B, L = 2, 2048
NCORES = 8
HPC = 4           # heads per core
FPC = HPC * DK    # features per core = 256
KD = D // 128     # 8 contraction subtiles for projections
LT, LTW = 4, 512  # l tiles
ST, STW = 16, 128  # s tiles
MPF = 3           # mask DMA prefetch depth
Ident = mybir.ActivationFunctionType.Identity
Exp = mybir.ActivationFunctionType.Exp

_CACHED_NC = None


def _build():
    nc = bacc.Bacc("TRN2", target_bir_lowering=False, debug=False,
                   num_devices=NCORES)
    xT = nc.declare_dram_parameter("xT", [128, KD, L], F16, isOutput=False)
    wq = nc.declare_dram_parameter("wq", [128, KD, FPC], F16, isOutput=False)
    wk = nc.declare_dram_parameter("wk", [128, KD, FPC], F16, isOutput=False)
    wv = nc.declare_dram_parameter("wv", [128, KD, FPC], F16, isOutput=False)
    wo = nc.declare_dram_parameter("wo", [128, 2, D], F16, isOutput=False)
    bq = nc.declare_dram_parameter("bq", [128, 2], F32, isOutput=False)
    bk = nc.declare_dram_parameter("bk", [128, 2], F32, isOutput=False)
    maskT = nc.declare_dram_parameter("maskT", [ST, LT, 128, LTW], F16,
                                      isOutput=False)
    out = nc.declare_dram_parameter("out", [128, ST, D], F16, isOutput=True)

    with tile.TileContext(nc) as tc, ExitStack() as ctx:
        pool = ctx.enter_context(tc.tile_pool(name="pers", bufs=1))
        mpool = ctx.enter_context(tc.tile_pool(name="mpool", bufs=2 * MPF))
        epool = ctx.enter_context(tc.tile_pool(name="epool", bufs=4))
        rbpool = ctx.enter_context(tc.tile_pool(name="rbpool", bufs=1))
        avpool = ctx.enter_context(tc.tile_pool(name="avpool", bufs=1))
        opool = ctx.enter_context(tc.tile_pool(name="opool", bufs=3))
        scp = ctx.enter_context(tc.tile_pool(name="scp", bufs=2, space="PSUM"))
        tp = ctx.enter_context(tc.tile_pool(name="tp", bufs=1, space="PSUM"))

        xt = pool.tile([128, KD, L], F16)
        wq_sb = pool.tile([128, KD, FPC], F16)
        wk_sb = pool.tile([128, KD, FPC], F16)
        wv_sb = pool.tile([128, KD, FPC], F16)
        wo_sb = pool.tile([128, 2, D], F16)
        bq_sb = pool.tile([128, 2], F32)
        bk_sb = pool.tile([128, 2], F32)
        # DMA issue order follows the dependency order of the first
        # matmuls.  x is transferred in l-quarters: the whole prefix (KT
        # chunk 0, QT l-chunk 0, V s-tiles 0-3) only touches sequence
        # positions 0:512, so the PE can start ~3.5us in instead of
        # waiting ~12us for all of x.
        nc.sync.dma_start(out=wk_sb[:], in_=wk[:])
        for kd in range(KD):
            nc.sync.dma_start(out=xt[:, kd, 0:LTW], in_=xT[:, kd, 0:LTW])
        nc.sync.dma_start(out=wv_sb[:], in_=wv[:])
        nc.sync.dma_start(out=wq_sb[:], in_=wq[:])
        nc.sync.dma_start(out=bk_sb[:], in_=bk[:])
        nc.sync.dma_start(out=bq_sb[:], in_=bq[:])
        for q in range(1, LT):
            qsl = slice(q * LTW, (q + 1) * LTW)
            for kd in range(KD):
                nc.sync.dma_start(out=xt[:, kd, qsl], in_=xT[:, kd, qsl])
        nc.sync.dma_start(out=wo_sb[:], in_=wo[:])

        # PE warmup: ~64 tiny matmuls on a zeroed tile keep the PE busy
        # during the initial DMA wait so the HAM clock-gate reaches 8/8
        # before the real projection matmuls start (cold MMs run at half
        # clock for the first ~3.4us of activity otherwise).
        wu_sb = pool.tile([128, 64], F16)
        nc.vector.memset(wu_sb[:], 0.0)
        wu_ps = scp.tile([128, 2, LTW], F32, tag="sc", name="wu")
        for i in range(40):
            nc.tensor.matmul(wu_ps[0:64, 0, 0:64], wu_sb[:], wu_sb[:],
                             start=True, stop=True)

        QT = pool.tile([128, 2, L], F16)   # [feat(2x128), l]: Q^T * 0.125
        KT = pool.tile([128, 2, L], F16)
        # Vaug[:, st, h]: even h -> [V_h | 1], odd h -> [1 | V_h]
        Vaug = pool.tile([128, ST, HPC, 128], F16)
        nc.gpsimd.memset(Vaug[:], 1.0)
        outTs = [pool.tile([128, 2, LTW], F16, name=f"outT{i}")
                 for i in range(LT)]

        # ---- producers, emitted in small quanta between attention s-tile
        # ---- iterations so the PE absorbs them while ACT (exp) streams.
        def emit_kt_half(c, ft):
            lsl = slice(c * LTW, (c + 1) * LTW)
            fsl = slice(ft * 128, (ft + 1) * 128)
            ps = scp.tile([128, 2, LTW], F32, tag="sc", name=f"pk{c}_{ft}")
            for kd in range(KD):
                nc.tensor.matmul(ps[:, 0, :], wk_sb[:, kd, fsl],
                                 xt[:, kd, lsl],
                                 start=(kd == 0), stop=(kd == KD - 1))
            nc.vector.scalar_tensor_tensor(
                KT[:, ft, lsl], ps[:, 0, :], 1.0,
                bk_sb[:, ft:ft + 1].to_broadcast((128, LTW)),
                mybir.AluOpType.mult, mybir.AluOpType.add)

        def emit_qt_half(lt, ft):
            lsl = slice(lt * LTW, (lt + 1) * LTW)
            fsl = slice(ft * 128, (ft + 1) * 128)
            ps = scp.tile([128, 2, LTW], F32, tag="sc", name=f"pq{lt}_{ft}")
            for kd in range(KD):
                nc.tensor.matmul(ps[:, 0, :], wq_sb[:, kd, fsl],
                                 xt[:, kd, lsl],
                                 start=(kd == 0), stop=(kd == KD - 1))
            nc.vector.scalar_tensor_tensor(
                QT[:, ft, lsl], ps[:, 0, :], 0.125,
                bq_sb[:, ft:ft + 1].to_broadcast((128, LTW)),
                mybir.AluOpType.mult, mybir.AluOpType.add)

        def emit_v_chunk(c):
            for st in range(4 * c, 4 * c + 4):
                ssl = slice(st * STW, (st + 1) * STW)
                psv = tp.tile([128, LTW], F32, tag=f"T{st % 4}",
                              name=f"psv{st}")
                for kd in range(KD):
                    nc.tensor.matmul(psv[:, :FPC], xt[:, kd, ssl],
                                     wv_sb[:, kd, :],
                                     start=(kd == 0), stop=(kd == KD - 1))
                for h in range(HPC):
                    off = 0 if h % 2 == 0 else 64
                    nc.vector.tensor_copy(Vaug[:, st, h, off:off + 64],
                                          psv[:, DK * h:DK * (h + 1)])

        def emit_outproj_group(lt8):
            ps3 = scp.tile([128, 2, LTW], F32, tag="sc", name=f"ps3_{lt8}")
            for nf in range(2):
                nsl = slice(nf * 512, (nf + 1) * 512)
                for pair in range(2):
                    nc.tensor.matmul(
                        ps3[:, nf, :],
                        outTs[lt8 // 4][:, pair,
                                        (lt8 % 4) * 128:(lt8 % 4 + 1) * 128],
                        wo_sb[:, pair, nsl],
                        start=(pair == 0), stop=(pair == 1))
            ob = opool.tile([128, D], F16)
            # evacuate the two PSUM banks on ACT and DVE in parallel so the
            # score-slot rotation frees ~2x sooner (tail pace is gated on it)
            nc.scalar.copy(ob[:, 0:512], ps3[:, 0, :])
            nc.vector.tensor_copy(ob[:, 512:1024], ps3[:, 1, :])
            nc.gpsimd.dma_start(out=out[:, lt8, :], in_=ob[:])

        # Injected work: QT for l-tile lt+1 is produced mid-lt (so no PE
        # burst at the boundary).  Everything else (KT, V) is produced in
        # the serial prefix below -- the attention phase has no PE slack to
        # absorb it (EXP-gated periods are ~95% PE-occupied already).
        inject = {}
        for lt in range(LT - 1):
            inject[(lt, 7)] = [lambda lt=lt: emit_qt_half(lt + 1, 0)]
            inject[(lt, 9)] = [lambda lt=lt: emit_qt_half(lt + 1, 1)]

        # ---- serial prefix: all projections, ordered to match DMA arrival
        # ---- of the x l-quarters (chunk c needs quarter c only).
        for c in range(4):
            emit_kt_half(c, 0)
            emit_kt_half(c, 1)
            if c == 0:
                emit_qt_half(0, 0)
                emit_qt_half(0, 1)
            emit_v_chunk(c)

        mk_tiles = {}
        mk_order = [(lt, st) for lt in range(LT) for st in range(ST)]

        def prefetch_mask(pos):
            if pos < len(mk_order):
                plt, pst = mk_order[pos]
                mk = mpool.tile([128, LTW], F16)
                nc.sync.dma_start(out=mk[:], in_=maskT[pst, plt])
                mk_tiles[(plt, pst)] = mk

        for pos in range(MPF):
            prefetch_mask(pos)

        # Per-head softmax normalization, split into three phases that are
        # injected into the NEXT l-tile's first s-iterations so the Tile
        # scheduler orders them after that l-tile's score/exp stream.  The
        # T PSUM banks are released by phase 1 (both halves of each bank
        # read out: av halves via ACT, which idles at the boundary, row
        # sums via DVE); the lane swaps, reciprocals and final multiplies
        # run off the critical path on SBUF staging tiles.
        rbs = [rbpool.tile([128, LTW], F32, name=f"rb{h}")
               for h in range(HPC)]
        avs = [avpool.tile([128, LTW], F32, name=f"av{h}")
               for h in range(HPC)]

        def chain_phase1a(Ts):
            for h in (1, 3):   # odd: av at 64:128, sums at 0:64
                nc.scalar.copy(avs[h][64:128, :], Ts[h][64:128, :])
                nc.vector.reciprocal_approx_fast(out=rbs[h][0:64, :],
                                                 in_=Ts[h][0:64, :])

        def chain_phase1b(Ts):
            for h in (0, 2):   # even: av at 0:64, sums at 64:128
                nc.scalar.copy(avs[h][0:64, :], Ts[h][0:64, :])
                nc.vector.tensor_copy(rbs[h][64:128, :], Ts[h][64:128, :])

        def chain_phase2a():
            # gpsimd queue: idle mid-stream, while Sync is busy with the
            # mask prefetches
            for h in (1, 3):
                nc.gpsimd.dma_start(out=rbs[h][64:128, :],
                                    in_=rbs[h][0:64, :])

        def chain_phase2b():
            for h in (0, 2):
                nc.gpsimd.dma_start(out=rbs[h][0:64, :],
                                    in_=rbs[h][64:128, :])

        def chain_phase1(Ts):
            chain_phase1a(Ts)
            chain_phase1b(Ts)

        def chain_phase2():
            chain_phase2a()
            chain_phase2b()

        def chain_phase3(lt):
            for h in (1, 3):
                nc.vector.tensor_mul(outTs[lt][64:128, h // 2, :],
                                     avs[h][64:128, :], rbs[h][64:128, :])
            for h in (0, 2):
                nc.vector.reciprocal_approx_fast(out=rbs[h][0:64, :],
                                                 in_=rbs[h][0:64, :])
                nc.vector.tensor_mul(outTs[lt][0:64, h // 2, :],
                                     avs[h][0:64, :], rbs[h][0:64, :])

        prevTs = None
        for lt in range(LT):
            lsl = slice(lt * LTW, (lt + 1) * LTW)
            Ts = [tp.tile([128, LTW], F32, tag=f"T{h}", name=f"T{h}_{lt}")
                  for h in range(HPC)]
            for st in range(ST):
                prefetch_mask(lt * ST + st + MPF)
                ssl = slice(st * STW, (st + 1) * STW)
                mk = mk_tiles.pop((lt, st))
                Es = []
                for pair in range(2):
                    sc = scp.tile([128, 2, LTW], F32, tag="sc")
                    for i in range(2):
                        nc.tensor.matmul(
                            sc[:, i, :],
                            KT[64 * i:64 * (i + 1), pair, ssl],
                            QT[64 * i:64 * (i + 1), pair, lsl],
                            start=True, stop=True)
                    E = epool.tile([128, 2, LTW], F16, name=f"E{pair}")
                    nc.scalar.activation(E[:], sc[:], Exp)
                    nc.vector.tensor_mul(
                        E[:], E[:],
                        mk[:, None, :].to_broadcast((128, 2, LTW)))
                    Es.append(E)
                # injected work sits between the score and aug matmuls in
                # emission (priority) order, filling the PE wait for the
                # exp+mask chain of this s-tile.
                if prevTs is not None:
                    if st == 0:
                        chain_phase1(prevTs)
                    elif st == 1:
                        chain_phase2()
                    elif st == 2:
                        chain_phase3(lt - 1)
                for fn in inject.get((lt, st), ()):
                    fn()
                # all four aug matmuls back-to-back: one weight-swap drain
                # boundary per s-tile instead of one per pair.  At st==0 the
                # odd heads go first: their T banks are released first by the
                # previous l-tile's phase-1 extraction.
                h_order = (1, 3, 0, 2) if st == 0 else (0, 1, 2, 3)
                for h in h_order:
                    pair, i = divmod(h, 2)
                    nc.tensor.matmul(Ts[h][:], Vaug[:, st, h, :],
                                     Es[pair][:, i, :],
                                     start=(st == 0), stop=(st == ST - 1))
            prevTs = Ts

        # tail: release the T banks immediately (phase 1), then stream all
        # 16 output-projection groups through them at single-bank
        # granularity (4-deep rotation, ACT/DVE alternating evacuations);
        # the rest of the last normalization (lane swaps, reciprocals,
        # multiplies) runs underneath the first groups and only the last 4
        # groups depend on it.
        # The first four groups go through the score PSUM slots, which are
        # released by the final exps BEFORE the last aug matmuls finish --
        # they overlap the end of the stream and the normalization chain.
        for lt8 in range(4):
            emit_outproj_group(lt8)
        chain_phase1(prevTs)

        def tail_outproj(lt8):
            ob = opool.tile([128, D], F16)
            for nf in range(2):
                nsl = slice(nf * 512, (nf + 1) * 512)
                pso = tp.tile([128, LTW], F32, tag=f"T{(2 * lt8 + nf) % 4}",
                              name=f"pso{lt8}_{nf}")
                for pair in range(2):
                    nc.tensor.matmul(
                        pso[:, :],
                        outTs[lt8 // 4][:, pair,
                                        (lt8 % 4) * 128:(lt8 % 4 + 1) * 128],
                        wo_sb[:, pair, nsl],
                        start=(pair == 0), stop=(pair == 1))
                if nf == 0:
                    nc.scalar.copy(ob[:, nsl], pso[:, :])
                else:
                    nc.vector.tensor_copy(ob[:, nsl], pso[:, :])
            if lt8 % 2 == 0:
                nc.gpsimd.dma_start(out=out[:, lt8, :], in_=ob[:])
            else:
                nc.sync.dma_start(out=out[:, lt8, :], in_=ob[:])

        for lt8 in range(4, 8):
            tail_outproj(lt8)
        chain_phase2()
        for lt8 in range(8, 12):
            tail_outproj(lt8)
        chain_phase3(LT - 1)
        for lt8 in range(12, 4 * LT):
            tail_outproj(lt8)

    nc.compile()
    return nc


def _get_nc():
    global _CACHED_NC
    if _CACHED_NC is None:
        _CACHED_NC = _build()
    return _CACHED_NC


def _prep_core_inputs(c, x, mask, Wq, bq, Wk, bk, Wv, Wo):
    b, g = divmod(c, 4)
    cs = slice(g * FPC, (g + 1) * FPC)

    xT = np.ascontiguousarray(
        x[b].T.reshape(KD, 128, L).transpose(1, 0, 2)).astype(np.float16)
    wq_c = np.ascontiguousarray(
        Wq[:, cs].reshape(KD, 128, FPC).transpose(1, 0, 2)).astype(np.float16)
    wk_c = np.ascontiguousarray(
        Wk[:, cs].reshape(KD, 128, FPC).transpose(1, 0, 2)).astype(np.float16)
    wv_c = np.ascontiguousarray(
        Wv[:, cs].reshape(KD, 128, FPC).transpose(1, 0, 2)).astype(np.float16)
    wo_c = np.ascontiguousarray(
        Wo[cs, :].reshape(2, 128, D).transpose(1, 0, 2)).astype(np.float16)
    bq_c = np.ascontiguousarray(
        (bq[cs] * 0.125).reshape(2, 128).T).astype(np.float32)
    bk_c = np.ascontiguousarray(bk[cs].reshape(2, 128).T).astype(np.float32)
    mT = mask[b].astype(np.float16).T  # [S, L]
    maskT = np.ascontiguousarray(
        mT.reshape(ST, 128, LT, LTW).transpose(0, 2, 1, 3))
    return {"xT": xT, "wq": wq_c, "wk": wk_c, "wv": wv_c, "wo": wo_c,
            "bq": bq_c, "bk": bk_c, "maskT": maskT}


def kernel(x, mask, Wq, bq, Wk, bk, Wv, bv, Wo, bo):
    x = np.asarray(x, np.float32)
    mask = np.asarray(mask)
    Wq, bq = np.asarray(Wq, np.float32), np.asarray(bq, np.float32)
    Wk, bk = np.asarray(Wk, np.float32), np.asarray(bk, np.float32)
    Wv, bv = np.asarray(Wv, np.float32), np.asarray(bv, np.float32)
    Wo, bo = np.asarray(Wo, np.float32), np.asarray(bo, np.float32)

    nc = _get_nc()
    in_maps = [_prep_core_inputs(c, x, mask, Wq, bq, Wk, bk, Wv, Wo)
               for c in range(NCORES)]
    res = run_bass_kernel_spmd(nc, in_maps, list(range(NCORES)))

    const_vec = (bv @ Wo + bo).astype(np.float32)  # A rows sum to 1
    outs = []
    for b in range(B):
        acc = np.zeros((L, D), np.float32)
        for g in range(4):
            part = res.results[4 * b + g]["out"]  # [128, 16, 1024] fp16
            acc += part.transpose(1, 0, 2).reshape(L, D).astype(np.float32)
        acc += const_vec
        outs.append(acc)
    return np.stack(outs)


# revision 32
# speedup vs baseline: 1.0495x; 1.0050x over previous
"""Multi-head attention (B=2, L=S=2048, D=1024, H=16) on 8 Trainium2 cores.

Sharding: core c -> batch b = c // 4, head group g = c % 4 (4 heads per core).
W_Q/K/V column-sharded (256 cols per core), W_O row-sharded (256 rows per core);
the 4 partial outputs per batch are summed on the host (plus bias terms).

Per-core pipeline (all big tensors kept transposed so no on-device transposes):
  projections: QT = 0.125*(x Wq + bq)^T, KT = (x Wk + bk)^T (feature-major
    [256, L]); Vaug = [V_h | ones] per head (seq-major, fp16), V bias folded
    out on the host (softmax rows sum to 1 => + bv @ Wo + bo once).
  attention, per (l-tile 512, s-tile 128): S^T = KT^T QT (row-packed pairs of
    heads, K=64); E = exp(S^T) * maskT (ACT exp from PSUM, per-pair 0/1 fp16
    mask multiply on DVE at 2x); T_h += Vaug_h^T E accumulates BOTH the head
    output AND its softmax row-sums in one full-array matmul (ones columns
    act as the reducer; even heads get [V|1] -> av in rows 0:64, odd heads
    [1|V] -> av in rows 64:128 so every result lands on the lanes the
    output-projection layout needs).
  out-projection: out_partial = outT^T Wo_rows (K=128, accumulate over the
    two 128-row groups).

Scheduling (the kernel is EXP-gated in steady state at ~1us/exp, with the
PE ~90% subscribed inside each period, so projections must NOT be pushed
into the attention stream):
  - serial prefix: all of KT/V/QT(l-tile 0), with x DMA'd in l-quarters so
    the PE starts ~4us in, plus ~40 tiny warm-up matmuls so the HAM clock
    gate reaches 8/8 before the real projections;
  - QT for l-tile lt+1 is the only producer injected mid-stream (2x 8
    matmuls borrowing the score PSUM rotation);
  - the per-l-tile softmax normalization is split into 3 phases injected
    into the NEXT l-tile's first s-iterations: phase 1 reads both halves of
    each T bank out of PSUM early (av halves on ACT, which idles at the
    boundary; row sums via DVE recip/copy) so the banks release ~1.5us
    after the last aug matmul; lane swaps (reciprocal_approx_fast only
    works at partition base 0) and the final multiplies run off the
    critical path on SBUF staging;
  - tail: the first 4 output-projection groups go through the score PSUM
    slots, which the final exps release BEFORE the last aug matmuls finish
    (they overlap the end of the stream and the normalization chain); then
    phase 1 of the last l-tile releases the T banks and the remaining 12
    groups stream through them at single-bank granularity (4-deep rotation;
    ACT/DVE alternate the PSUM evacuations, both idle after the last exp).

All matmul operands fp16 (1 cyc/row, no packing restrictions); PSUM fp32.
PSUM budget 8 banks = scores 2x2 + T_h 4x1; projections borrow the score
slots, the tail out-projection borrows the T banks.
"""
from contextlib import ExitStack

import numpy as np

import concourse.bass as bass
import concourse.mybir as mybir
import concourse.tile as tile
from concourse import bacc
from concourse.bass_utils import run_bass_kernel_spmd

F16 = mybir.dt.float16
F32 = mybir.dt.float32

D = 1024          # d_model
H = 16            # heads
DK = 64           # head dim
B, L = 2, 2048
NCORES = 8
HPC = 4           # heads per core
FPC = HPC * DK    # features per core = 256
KD = D // 128     # 8 contraction subtiles for projections
LT, LTW = 4, 512  # l tiles
ST, STW = 16, 128  # s tiles
MPF = 3           # mask DMA prefetch depth
Ident = mybir.ActivationFunctionType.Identity
Exp = mybir.ActivationFunctionType.Exp

_CACHED_NC = None


def _build():
    nc = bacc.Bacc("TRN2", target_bir_lowering=False, debug=False,
                   num_devices=NCORES)
    xT = nc.declare_dram_parameter("xT", [128, KD, L], F16, isOutput=False)
    wq = nc.declare_dram_parameter("wq", [128, KD, FPC], F16, isOutput=False)
    wk = nc.declare_dram_parameter("wk", [128, KD, FPC], F16, isOutput=False)
    wv = nc.declare_dram_parameter("wv", [128, KD, FPC], F16, isOutput=False)
    wo = nc.declare_dram_parameter("wo", [128, 2, D], F16, isOutput=False)
    bq = nc.declare_dram_parameter("bq", [128, 2], F32, isOutput=False)
    bk = nc.declare_dram_parameter("bk", [128, 2], F32, isOutput=False)
    maskT = nc.declare_dram_parameter("maskT", [ST, LT, 128, LTW], F16,
                                      isOutput=False)
    out = nc.declare_dram_parameter("out", [128, ST, D], F16, isOutput=True)

    with tile.TileContext(nc) as tc, ExitStack() as ctx:
        pool = ctx.enter_context(tc.tile_pool(name="pers", bufs=1))
        mpool = ctx.enter_context(tc.tile_pool(name="mpool", bufs=2 * MPF))
        epool = ctx.enter_context(tc.tile_pool(name="epool", bufs=4))
        rbpool = ctx.enter_context(tc.tile_pool(name="rbpool", bufs=1))
        avpool = ctx.enter_context(tc.tile_pool(name="avpool", bufs=1))
        opool = ctx.enter_context(tc.tile_pool(name="opool", bufs=3))
        scp = ctx.enter_context(tc.tile_pool(name="scp", bufs=2, space="PSUM"))
        tp = ctx.enter_context(tc.tile_pool(name="tp", bufs=1, space="PSUM"))

        xt = pool.tile([128, KD, L], F16)
        wq_sb = pool.tile([128, KD, FPC], F16)
        wk_sb = pool.tile([128, KD, FPC], F16)
        wv_sb = pool.tile([128, KD, FPC], F16)
        wo_sb = pool.tile([128, 2, D], F16)
        bq_sb = pool.tile([128, 2], F32)
        bk_sb = pool.tile([128, 2], F32)
        # DMA issue order follows the dependency order of the first
        # matmuls.  x is transferred in l-quarters: the whole prefix (KT
        # chunk 0, QT l-chunk 0, V s-tiles 0-3) only touches sequence
        # positions 0:512, so the PE can start ~3.5us in instead of
        # waiting ~12us for all of x.
        nc.sync.dma_start(out=wk_sb[:], in_=wk[:])
        for kd in range(KD):
            nc.sync.dma_start(out=xt[:, kd, 0:LTW], in_=xT[:, kd, 0:LTW])
        nc.sync.dma_start(out=wv_sb[:], in_=wv[:])
        nc.sync.dma_start(out=wq_sb[:], in_=wq[:])
        nc.sync.dma_start(out=bk_sb[:], in_=bk[:])
        nc.sync.dma_start(out=bq_sb[:], in_=bq[:])
        for q in range(1, LT):
            qsl = slice(q * LTW, (q + 1) * LTW)
            for kd in range(KD):
                nc.sync.dma_start(out=xt[:, kd, qsl], in_=xT[:, kd, qsl])
        nc.sync.dma_start(out=wo_sb[:], in_=wo[:])

        # PE warmup: ~64 tiny matmuls on a zeroed tile keep the PE busy
        # during the initial DMA wait so the HAM clock-gate reaches 8/8
        # before the real projection matmuls start (cold MMs run at half
        # clock for the first ~3.4us of activity otherwise).
        wu_sb = pool.tile([128, 64], F16)
        nc.vector.memset(wu_sb[:], 0.0)
        wu_ps = scp.tile([128, 2, LTW], F32, tag="sc", name="wu")
        for i in range(40):
            nc.tensor.matmul(wu_ps[0:64, 0, 0:64], wu_sb[:], wu_sb[:],
                             start=True, stop=True)

        QT = pool.tile([128, 2, L], F16)   # [feat(2x128), l]: Q^T * 0.125
        KT = pool.tile([128, 2, L], F16)
        # Vaug[:, st, h]: even h -> [V_h | 1], odd h -> [1 | V_h]
        Vaug = pool.tile([128, ST, HPC, 128], F16)
        nc.gpsimd.memset(Vaug[:], 1.0)
        outTs = [pool.tile([128, 2, LTW], F16, name=f"outT{i}")
                 for i in range(LT)]

        # ---- producers, emitted in small quanta between attention s-tile
        # ---- iterations so the PE absorbs them while ACT (exp) streams.
        def emit_kt_half(c, ft):
            lsl = slice(c * LTW, (c + 1) * LTW)
            fsl = slice(ft * 128, (ft + 1) * 128)
            ps = scp.tile([128, 2, LTW], F32, tag="sc", name=f"pk{c}_{ft}")
            for kd in range(KD):
                nc.tensor.matmul(ps[:, 0, :], wk_sb[:, kd, fsl],
                                 xt[:, kd, lsl],
                                 start=(kd == 0), stop=(kd == KD - 1))
            nc.vector.scalar_tensor_tensor(
                KT[:, ft, lsl], ps[:, 0, :], 1.0,
                bk_sb[:, ft:ft + 1].to_broadcast((128, LTW)),
                mybir.AluOpType.mult, mybir.AluOpType.add)

        def emit_qt_half(lt, ft):
            lsl = slice(lt * LTW, (lt + 1) * LTW)
            fsl = slice(ft * 128, (ft + 1) * 128)
            ps = scp.tile([128, 2, LTW], F32, tag="sc", name=f"pq{lt}_{ft}")
            for kd in range(KD):
                nc.tensor.matmul(ps[:, 0, :], wq_sb[:, kd, fsl],
                                 xt[:, kd, lsl],
                                 start=(kd == 0), stop=(kd == KD - 1))
            nc.vector.scalar_tensor_tensor(
                QT[:, ft, lsl], ps[:, 0, :], 0.125,
                bq_sb[:, ft:ft + 1].to_broadcast((128, LTW)),
                mybir.AluOpType.mult, mybir.AluOpType.add)

        def emit_v_chunk(c):
            for st in range(4 * c, 4 * c + 4):
                ssl = slice(st * STW, (st + 1) * STW)
                psv = tp.tile([128, LTW], F32, tag=f"T{st % 4}",
                              name=f"psv{st}")
                for kd in range(KD):
                    nc.tensor.matmul(psv[:, :FPC], xt[:, kd, ssl],
                                     wv_sb[:, kd, :],
                                     start=(kd == 0), stop=(kd == KD - 1))
                for h in range(HPC):
                    off = 0 if h % 2 == 0 else 64
                    nc.vector.tensor_copy(Vaug[:, st, h, off:off + 64],
                                          psv[:, DK * h:DK * (h + 1)])

        def emit_outproj_group(lt8):
            ps3 = scp.tile([128, 2, LTW], F32, tag="sc", name=f"ps3_{lt8}")
            for nf in range(2):
                nsl = slice(nf * 512, (nf + 1) * 512)
                for pair in range(2):
                    nc.tensor.matmul(
                        ps3[:, nf, :],
                        outTs[lt8 // 4][:, pair,
                                        (lt8 % 4) * 128:(lt8 % 4 + 1) * 128],
                        wo_sb[:, pair, nsl],
                        start=(pair == 0), stop=(pair == 1))
            ob = opool.tile([128, D], F16)
            # evacuate the two PSUM banks on ACT and DVE in parallel so the
            # score-slot rotation frees ~2x sooner (tail pace is gated on it)
            nc.scalar.copy(ob[:, 0:512], ps3[:, 0, :])
            nc.vector.tensor_copy(ob[:, 512:1024], ps3[:, 1, :])
            nc.gpsimd.dma_start(out=out[:, lt8, :], in_=ob[:])

        # Injected work: QT for l-tile lt+1 is produced mid-lt (so no PE
        # burst at the boundary).  Everything else (KT, V) is produced in
        # the serial prefix below -- the attention phase has no PE slack to
        # absorb it (EXP-gated periods are ~95% PE-occupied already).
        inject = {}
        for lt in range(LT - 1):
            inject[(lt, 7)] = [lambda lt=lt: emit_qt_half(lt + 1, 0)]
            inject[(lt, 9)] = [lambda lt=lt: emit_qt_half(lt + 1, 1)]

        # ---- serial prefix: all projections, ordered to match DMA arrival
        # ---- of the x l-quarters (chunk c needs quarter c only).
        for c in range(4):
            emit_kt_half(c, 0)
            emit_kt_half(c, 1)
            if c == 0:
                emit_qt_half(0, 0)
                emit_qt_half(0, 1)
            emit_v_chunk(c)

        mk_tiles = {}
        mk_order = [(lt, st) for lt in range(LT) for st in range(ST)]

        def prefetch_mask(pos):
            # masks are fetched in s-tile pairs sharing one tile: half the
            # pool allocations (and their teardown semaphores) for the same
            # DMA traffic
            if pos < len(mk_order) and pos % 2 == 0:
                plt, pst = mk_order[pos]
                mk2 = mpool.tile([128, 2, LTW], F16)
                nc.sync.dma_start(out=mk2[:, 0, :], in_=maskT[pst, plt])
                nc.sync.dma_start(out=mk2[:, 1, :], in_=maskT[pst + 1, plt])
                mk_tiles[(plt, pst)] = mk2[:, 0, :]
                mk_tiles[(plt, pst + 1)] = mk2[:, 1, :]

        for pos in range(MPF + 1):
            prefetch_mask(pos)

        # Per-head softmax normalization, split into three phases that are
        # injected into the NEXT l-tile's first s-iterations so the Tile
        # scheduler orders them after that l-tile's score/exp stream.  The
        # T PSUM banks are released by phase 1 (both halves of each bank
        # read out: av halves via ACT, which idles at the boundary, row
        # sums via DVE); the lane swaps, reciprocals and final multiplies
        # run off the critical path on SBUF staging tiles.
        rbs = [rbpool.tile([128, LTW], F32, name=f"rb{h}")
               for h in range(HPC)]
        avs = [avpool.tile([128, LTW], F32, name=f"av{h}")
               for h in range(HPC)]

        def chain_phase1a(Ts):
            for h in (1, 3):   # odd: av at 64:128, sums at 0:64
                nc.scalar.copy(avs[h][64:128, :], Ts[h][64:128, :])
                nc.vector.reciprocal_approx_fast(out=rbs[h][0:64, :],
                                                 in_=Ts[h][0:64, :])

        def chain_phase1b(Ts):
            for h in (0, 2):   # even: av at 0:64, sums at 64:128
                nc.scalar.copy(avs[h][0:64, :], Ts[h][0:64, :])
                nc.vector.tensor_copy(rbs[h][64:128, :], Ts[h][64:128, :])

        def chain_phase2a():
            # gpsimd queue: idle mid-stream, while Sync is busy with the
            # mask prefetches
            for h in (1, 3):
                nc.gpsimd.dma_start(out=rbs[h][64:128, :],
                                    in_=rbs[h][0:64, :])

        def chain_phase2b():
            for h in (0, 2):
                nc.gpsimd.dma_start(out=rbs[h][0:64, :],
                                    in_=rbs[h][64:128, :])

        def chain_phase1(Ts):
            chain_phase1a(Ts)
            chain_phase1b(Ts)

        def chain_phase2():
            chain_phase2a()
            chain_phase2b()

        def chain_phase3(lt):
            for h in (1, 3):
                nc.vector.tensor_mul(outTs[lt][64:128, h // 2, :],
                                     avs[h][64:128, :], rbs[h][64:128, :])
            for h in (0, 2):
                nc.vector.reciprocal_approx_fast(out=rbs[h][0:64, :],
                                                 in_=rbs[h][0:64, :])
                nc.vector.tensor_mul(outTs[lt][0:64, h // 2, :],
                                     avs[h][0:64, :], rbs[h][0:64, :])

        prevTs = None
        for lt in range(LT):
            lsl = slice(lt * LTW, (lt + 1) * LTW)
            Ts = [tp.tile([128, LTW], F32, tag=f"T{h}", name=f"T{h}_{lt}")
                  for h in range(HPC)]
            for st in range(ST):
                prefetch_mask(lt * ST + st + MPF)
                ssl = slice(st * STW, (st + 1) * STW)
                mk = mk_tiles.pop((lt, st))
                Es = []
                for pair in range(2):
                    sc = scp.tile([128, 2, LTW], F32, tag="sc")
                    for i in range(2):
                        nc.tensor.matmul(
                            sc[:, i, :],
                            KT[64 * i:64 * (i + 1), pair, ssl],
                            QT[64 * i:64 * (i + 1), pair, lsl],
                            start=True, stop=True)
                    E = epool.tile([128, 2, LTW], F16, name=f"E{pair}")
                    nc.scalar.activation(E[:], sc[:], Exp)
                    nc.vector.tensor_mul(
                        E[:], E[:],
                        mk[:, None, :].to_broadcast((128, 2, LTW)))
                    Es.append(E)
                # injected work sits between the score and aug matmuls in
                # emission (priority) order, filling the PE wait for the
                # exp+mask chain of this s-tile.
                if prevTs is not None:
                    if st == 0:
                        chain_phase1(prevTs)
                    elif st == 1:
                        chain_phase2()
                    elif st == 2:
                        chain_phase3(lt - 1)
                for fn in inject.get((lt, st), ()):
                    fn()
                # all four aug matmuls back-to-back: one weight-swap drain
                # boundary per s-tile instead of one per pair.  At st==0 the
                # odd heads go first: their T banks are released first by the
                # previous l-tile's phase-1 extraction.
                h_order = (1, 3, 0, 2) if st == 0 else (0, 1, 2, 3)
                for h in h_order:
                    pair, i = divmod(h, 2)
                    nc.tensor.matmul(Ts[h][:], Vaug[:, st, h, :],
                                     Es[pair][:, i, :],
                                     start=(st == 0), stop=(st == ST - 1))
            prevTs = Ts

        # tail: release the T banks immediately (phase 1), then stream all
        # 16 output-projection groups through them at single-bank
        # granularity (4-deep rotation, ACT/DVE alternating evacuations);
        # the rest of the last normalization (lane swaps, reciprocals,
        # multiplies) runs underneath the first groups and only the last 4
        # groups depend on it.
        # The first four groups go through the score PSUM slots, which are
        # released by the final exps BEFORE the last aug matmuls finish --
        # they overlap the end of the stream and the normalization chain.
        for lt8 in range(4):
            emit_outproj_group(lt8)
        chain_phase1(prevTs)

        def tail_outproj(lt8):
            ob = opool.tile([128, D], F16)
            for nf in range(2):
                nsl = slice(nf * 512, (nf + 1) * 512)
                pso = tp.tile([128, LTW], F32, tag=f"T{(2 * lt8 + nf) % 4}",
                              name=f"pso{lt8}_{nf}")
                for pair in range(2):
                    nc.tensor.matmul(
                        pso[:, :],
                        outTs[lt8 // 4][:, pair,
                                        (lt8 % 4) * 128:(lt8 % 4 + 1) * 128],
                        wo_sb[:, pair, nsl],
                        start=(pair == 0), stop=(pair == 1))
                if nf == 0:
                    nc.scalar.copy(ob[:, nsl], pso[:, :])
                else:
                    nc.vector.tensor_copy(ob[:, nsl], pso[:, :])
            if lt8 % 2 == 0:
                nc.gpsimd.dma_start(out=out[:, lt8, :], in_=ob[:])
            else:
                nc.sync.dma_start(out=out[:, lt8, :], in_=ob[:])

        for lt8 in range(4, 8):
            tail_outproj(lt8)
        chain_phase2()
        for lt8 in range(8, 12):
            tail_outproj(lt8)
        chain_phase3(LT - 1)
        for lt8 in range(12, 4 * LT):
            tail_outproj(lt8)

    nc.compile()
    return nc


def _get_nc():
    global _CACHED_NC
    if _CACHED_NC is None:
        _CACHED_NC = _build()
    return _CACHED_NC


def _prep_core_inputs(c, x, mask, Wq, bq, Wk, bk, Wv, Wo):
    b, g = divmod(c, 4)
    cs = slice(g * FPC, (g + 1) * FPC)

    xT = np.ascontiguousarray(
        x[b].T.reshape(KD, 128, L).transpose(1, 0, 2)).astype(np.float16)
    wq_c = np.ascontiguousarray(
        Wq[:, cs].reshape(KD, 128, FPC).transpose(1, 0, 2)).astype(np.float16)
    wk_c = np.ascontiguousarray(
        Wk[:, cs].reshape(KD, 128, FPC).transpose(1, 0, 2)).astype(np.float16)
    wv_c = np.ascontiguousarray(
        Wv[:, cs].reshape(KD, 128, FPC).transpose(1, 0, 2)).astype(np.float16)
    wo_c = np.ascontiguousarray(
        Wo[cs, :].reshape(2, 128, D).transpose(1, 0, 2)).astype(np.float16)
    bq_c = np.ascontiguousarray(
        (bq[cs] * 0.125).reshape(2, 128).T).astype(np.float32)
    bk_c = np.ascontiguousarray(bk[cs].reshape(2, 128).T).astype(np.float32)
    mT = mask[b].astype(np.float16).T  # [S, L]
    maskT = np.ascontiguousarray(
        mT.reshape(ST, 128, LT, LTW).transpose(0, 2, 1, 3))
    return {"xT": xT, "wq": wq_c, "wk": wk_c, "wv": wv_c, "wo": wo_c,
            "bq": bq_c, "bk": bk_c, "maskT": maskT}


def kernel(x, mask, Wq, bq, Wk, bk, Wv, bv, Wo, bo):
    x = np.asarray(x, np.float32)
    mask = np.asarray(mask)
    Wq, bq = np.asarray(Wq, np.float32), np.asarray(bq, np.float32)
    Wk, bk = np.asarray(Wk, np.float32), np.asarray(bk, np.float32)
    Wv, bv = np.asarray(Wv, np.float32), np.asarray(bv, np.float32)
    Wo, bo = np.asarray(Wo, np.float32), np.asarray(bo, np.float32)

    nc = _get_nc()
    in_maps = [_prep_core_inputs(c, x, mask, Wq, bq, Wk, bk, Wv, Wo)
               for c in range(NCORES)]
    res = run_bass_kernel_spmd(nc, in_maps, list(range(NCORES)))

    const_vec = (bv @ Wo + bo).astype(np.float32)  # A rows sum to 1
    outs = []
    for b in range(B):
        acc = np.zeros((L, D), np.float32)
        for g in range(4):
            part = res.results[4 * b + g]["out"]  # [128, 16, 1024] fp16
            acc += part.transpose(1, 0, 2).reshape(L, D).astype(np.float32)
        acc += const_vec
        outs.append(acc)
    return np.stack(outs)
